# revision 32
# baseline (speedup 1.0000x reference)
"""AttnBlock (LayerNorm + single-head self-attention + proj + residual) on 8
Trainium2 NeuronCores.

Problem: x [4, 512, 64, 64] f32; per batch image: t = LN(x) over channels;
qkv = t @ w_qkv.T; attn = softmax(q k^T / sqrt(c)); out = attn v @ w_proj.T;
y = x + out.

Sharding: 8 cores = 4 batches x 2 query-halves. Each core gets its batch's
full image (token order rolled so its 2048 queries are local tokens 0..2047),
computes LN + K/V over all 4096 tokens and Q over its half, then
scores/softmax/attn-V/proj for its 2048 queries. No collectives.

v4 design notes (vs v3):
  - Phase B pipeline deepened to 3 steps (stats -> LN apply -> QKV) so the
    serial stats->broadcast->apply chain (~7us with sem hops) never stalls
    the PE's QKV matmuls.
  - The -mu/rstd broadcast is two chained DMAs (row to DRAM, then a
    partition-broadcast read back): no PE matmuls, no DVE psum-copy.
  - LN stats: fp8 DR ones-matmuls over host-marshaled fp8 pair tiles of x
    and x^2; block 0's tile is split into 4 chunk DMAs so the first stats
    matmul starts after 128KB, not 512KB.
  - LN apply: za = x - mu (DVE 2x), then one scalar_tensor_tensor
    (za * gamma_col) * rstd_bc -> fp8; two of the four casts per block run
    on GPSIMD. beta==0 fast path (general beta build falls back).
  - Evictions: K^T/Q^T on ACT, V split ACT/DVE (gpsimd can't read PSUM).
  - Softmax denominator: DVE accumulates exp pairs u<=13 (cast early);
    pairs 14/15 are reduced by two DR ones-matmuls straight into the den
    PSUM, so the last tail never waits on the serial DVE add chain.
    es pool holds 4 exp pairs so score prefetch doesn't block on DVE lag.
  - fin STT + output DMA split into half-rows; out/xr/xbf DMAs issue on
    the SP hardware DGE (gpsimd keeps the 5D fp8 stat loads).
"""
import numpy as np

import concourse.bass as bass
import concourse.tile as tile
from concourse import mybir
from concourse.bass_utils import run_bass_kernel_spmd

P = 128
C = 512          # channels
T = 4096         # tokens per image
TQ = 2048        # queries per core
CB = C // P      # 4 channel chunks
W2 = CB // 2     # 2 channel chunk-pairs
TBLK = 512       # token block for LN/QKV phase
NTB = T // TBLK  # 8
NQB = TQ // TBLK  # 4 query blocks
NKT = T // P     # 32 key chunks
F32 = mybir.dt.float32
BF16 = mybir.dt.bfloat16
FP8 = mybir.dt.float8e4
FP = mybir.ActivationFunctionType
ALU = mybir.AluOpType
DR = mybir.MatmulPerfMode.DoubleRow
SCALE = float(C) ** -0.5
WS = 8.0          # fp8 weight prescale


def split_multiwaits(nc, max_waits=1):
    """walrus codegen allows one sync-wait slot on most TPB instruction
    structs; Tile's sem assignment emits several. Split extras into
    wait-only EventSemaphore instructions on the same engine stream."""
    n = 0
    for fn in nc.m.functions:
        for blk in fn.blocks:
            out = []
            for inst in blk.instructions:
                si = inst.sync_info
                if si is not None and si.on_wait is not None and len(si.on_wait) > max_waits:
                    extra = list(si.on_wait[:-max_waits])
                    keep = list(si.on_wait[-max_waits:])
                    for w in extra:
                        ev = mybir.InstEventSemaphore(
                            name=nc.get_next_instruction_name(),
                            engine=inst.engine,
                            sync_info=mybir.SyncInfo(on_wait=[w], on_update=[]),
                        )
                        out.append(ev)
                        n += 1
                    si.on_wait = keep
                out.append(inst)
            blk.instructions[:] = out
    return n


def build_nc(beta_zero=True, gamma_one=True, z_gpsimd=True, bc_dma=False):
    """beta_zero: ln_beta==0 (host-verified) - LN apply fuses gamma*rstd
    into one STT writing fp8. gamma_one: ln_gamma==1 as well - the STT
    becomes a plain tensor_mul, which Pool supports (TensorScalarPtr is
    rejected on Pool), so two of four casts can run on GPSIMD (z_gpsimd).
    bc_dma: broadcast -mu/rstd via DRAM bounce instead of PE matmuls."""
    nc = bass.Bass()
    xbf = nc.declare_dram_parameter("xbf", [C, T], BF16, isOutput=False)
    # tb-major fp8 stat tiles: each [:, tb] block is per-partition contiguous
    # (4KB), so the loads are 2D/3D patterns the SP hardware DGE can issue -
    # gpsimd's queue stays empty for the LN-apply multiplies
    x8q = nc.declare_dram_parameter("x8q", [P, NTB, W2, 2, 2, TBLK], FP8,
                                    isOutput=False)
    xres = nc.declare_dram_parameter("xres", [TQ, C], F32, isOutput=False)
    wq8d = nc.declare_dram_parameter("wq8d", [W2, P, 2, 3 * C], FP8, isOutput=False)
    wp8d = nc.declare_dram_parameter("wp8d", [W2, P, 2, C], FP8, isOutput=False)
    gamma = nc.declare_dram_parameter("gamma", [C], F32, isOutput=False)
    beta = nc.declare_dram_parameter("beta", [C], F32, isOutput=False)
    out = nc.declare_dram_parameter("out", [TQ, C], F32, isOutput=True)
    srow_d = nc.dram_tensor("srow_d", [NTB, 2, TBLK], BF16)

    with tile.TileContext(nc) as tc:
        with (
            tc.tile_pool(name="xs", bufs=3) as xs,
            tc.tile_pool(name="x8s", bufs=3) as x8s,
            tc.tile_pool(name="consts", bufs=1) as consts,
            tc.tile_pool(name="resid", bufs=1) as resid,
        ):
            # first gpsimd issues: block 0's fp8 stat tile, split by
            # (w, x|sq) chunk so the first stats matmul waits on 128KB
            x8_0 = consts.tile([P, W2, 2, 2, TBLK], FP8, tag="x8_0", name="x8_0")
            for w in range(W2):
                for s in range(2):
                    nc.sync.dma_start(out=x8_0[:, w, :, s, :],
                                      in_=x8q[:, 0, w, :, s, :])
            xb0 = consts.tile([P, CB, TBLK], BF16, tag="xb0", name="xb0")
            nc.sync.dma_start(
                out=xb0, in_=xbf[:, 0:TBLK].rearrange("(cb p) t -> p cb t", cb=CB))
            x8_1 = x8s.tile([P, W2, 2, 2, TBLK], FP8, tag="x8", name="x8_1pre")
            nc.sync.dma_start(out=x8_1, in_=x8q[:, 1])
            xb1_pre = xs.tile([P, CB, TBLK], BF16, tag="xb", name="xb1_pre")
            nc.sync.dma_start(
                out=xb1_pre,
                in_=xbf[:, TBLK:2 * TBLK].rearrange("(cb p) t -> p cb t", cb=CB))
            # region-split weight loads: the K block is what the first QKV
            # step consumes first - land it before the x prefetches queue up
            wq8 = []
            for w in range(W2):
                t = consts.tile([P, 2, 3 * C], FP8, tag=f"wq8_{w}", name=f"wq8_{w}")
                nc.sync.dma_start(out=t[:, :, C:2 * C], in_=wq8d[w][:, :, C:2 * C])
                wq8.append(t)
            for w in range(W2):
                nc.sync.dma_start(out=wq8[w][:, :, 2 * C:3 * C],
                                  in_=wq8d[w][:, :, 2 * C:3 * C])
            for w in range(W2):
                nc.sync.dma_start(out=wq8[w][:, :, 0:C], in_=wq8d[w][:, :, 0:C])
            # ---- constants ----
            gcolt = consts.tile([P, CB], F32, tag="gcolt")
            nc.sync.dma_start(out=gcolt, in_=gamma.rearrange("(cb p) -> p cb", cb=CB))
            bcolt = consts.tile([P, CB], F32, tag="bcolt")
            nc.sync.dma_start(out=bcolt, in_=beta.rearrange("(cb p) -> p cb", cb=CB))
            wp8 = []
            for w in range(W2):
                t = consts.tile([P, 2, C], FP8, tag=f"wp8_{w}", name=f"wp8_{w}")
                nc.sync.dma_start(out=t, in_=wp8d[w])
                wp8.append(t)
            ones_col_bf = consts.tile([P, 1], BF16, tag="ones_col_bf")
            nc.vector.memset(ones_col_bf, 1.0)
            ones_row = consts.tile([1, P], BF16, tag="ones_row")
            nc.vector.memset(ones_row, 1.0)
            # stats/den stationary: [128,2,1] slice of a [128,2,16] tile
            # (dual-fp8 LDWEIGHTS rejects a free-standing [128,2,1] with
            # tiny row stride; 16-elem pair stride satisfies step%16==0)
            ones_pair_w = consts.tile([P, 2, 16], FP8, tag="ones_pair_w")
            nc.vector.memset(ones_pair_w, 1.0)
            ones_pair8 = ones_pair_w[:, :, 0:1]
            eps_t = consts.tile([1, 1], F32, tag="eps_t")
            nc.vector.memset(eps_t, 1e-5)
            ones11 = consts.tile([1, 1], F32, tag="ones11")
            nc.vector.memset(ones11, 1.0)
            neg2 = consts.tile([P, 1], F32, tag="neg2")
            nc.vector.memset(neg2, -2.0)

            # ---- resident tensors ----
            KT = []   # K^T pairs: 2 x [128, 2, 4096] fp8 (DoubleRow layout)
            for w in range(W2):
                KT.append(resid.tile([P, 2, T], FP8, tag=f"KT{w}", name=f"KT{w}"))
            V = []    # V [tokenpair, d]: 16 x [128, 2, 512] fp8 (DoubleRow layout)
            for u in range(NKT // 2):
                V.append(resid.tile([P, 2, C], FP8, tag=f"V{u}", name=f"V{u}"))
            QT = []   # resident Q: per qb, per w: [128, 2, 512] fp8
            for qb in range(NQB):
                QT.append([resid.tile([P, 2, TBLK], FP8, tag=f"QT{qb}_{w}",
                                      name=f"QT{qb}_{w}") for w in range(W2)])

            # =========== Phase B: LN + QKV ===========
            with (
                tc.tile_pool(name="stat", bufs=1) as stat,
                tc.tile_pool(name="rows", bufs=2) as rows,
                tc.tile_pool(name="bcs", bufs=3) as bcs,
                tc.tile_pool(name="lns", bufs=4) as lns,
                tc.tile_pool(name="ztmp", bufs=3) as ztmp,
                tc.tile_pool(name="ps_row", bufs=1, space="PSUM") as ps_row,
                tc.tile_pool(name="ps_q", bufs=1, space="PSUM") as ps_q,
            ):
                statrow = [None] * NTB   # [1,2,TBLK] bf16: (-mu, rstd) rows
                bc_t = [None] * NTB
                xb_t = [None] * NTB
                x8_t = [None] * NTB
                q_slot = [0]

                def pair_psum(prefix, tb):
                    tag = f"pq{q_slot[0] % 3}"
                    q_slot[0] += 1
                    return ps_q.tile([P, 2, TBLK], F32, tag=tag,
                                     name=f"{prefix}{tb}")

                def load_xb(tb):
                    if tb >= NTB or xb_t[tb] is not None:
                        return
                    if tb == 0:
                        xb_t[tb] = xb0
                        x8_t[tb] = x8_0
                        return
                    if tb == 1:
                        xb_t[tb] = xb1_pre
                        x8_t[tb] = x8_1
                        return
                    ts = slice(tb * TBLK, (tb + 1) * TBLK)
                    x8 = x8s.tile([P, W2, 2, 2, TBLK], FP8, tag="x8",
                                  name=f"x8_{tb}")
                    nc.sync.dma_start(out=x8, in_=x8q[:, tb])
                    x8_t[tb] = x8
                    xb = xs.tile([P, CB, TBLK], BF16, tag="xb", name=f"xb{tb}")
                    nc.sync.dma_start(
                        out=xb, in_=xbf[:, ts].rearrange("(cb p) t -> p cb t", cb=CB))
                    xb_t[tb] = xb

                def b1_block(tb):
                    load_xb(tb)
                    load_xb(tb + 1)  # prefetch next block's x a step early
                    x8 = x8_t[tb]
                    # stats: fp8 DR ones-matmuls (contraction 256/pass)
                    s1 = ps_row.tile([1, TBLK], F32, tag="s1", name=f"s1_{tb}")
                    for w in range(W2):
                        nc.tensor.matmul(s1, ones_pair8, x8[:, w, :, 0, :],
                                         perf_mode=DR,
                                         start=(w == 0), stop=(w == W2 - 1))
                    s2 = ps_row.tile([1, TBLK], F32, tag="s2", name=f"s2_{tb}")
                    for w in range(W2):
                        nc.tensor.matmul(s2, ones_pair8, x8[:, w, :, 1, :],
                                         perf_mode=DR,
                                         start=(w == 0), stop=(w == W2 - 1))
                    # -mu row (ACT), then mu^2 and var on GPSIMD (immediate
                    # scalars only - Pool rejects scalar pointers), keeping
                    # DVE free for the LN apply and evictions
                    sr = stat.tile([1, 2, TBLK], BF16, tag=f"statrow{tb}",
                                   name=f"statrow{tb}")
                    nc.scalar.activation(out=sr[:, 0, :], in_=s1, func=FP.Copy,
                                         scale=-1.0 / C)
                    musq = rows.tile([1, TBLK], F32, tag="musq", name=f"musq{tb}")
                    nc.vector.tensor_mul(out=musq, in0=sr[:, 0, :], in1=sr[:, 0, :])
                    var = rows.tile([1, TBLK], F32, tag="var", name=f"var{tb}")
                    nc.vector.scalar_tensor_tensor(
                        out=var, in0=s2, scalar=1.0 / C, in1=musq,
                        op0=ALU.mult, op1=ALU.subtract)
                    lnv = rows.tile([1, TBLK], F32, tag="lnv", name=f"lnv{tb}")
                    nc.scalar.activation(out=lnv, in_=var, func=FP.Ln, bias=eps_t)
                    nc.scalar.activation(out=sr[:, 1, :], in_=lnv, func=FP.Exp,
                                         scale=-0.5)
                    statrow[tb] = sr

                def bc_block(tb):
                    # broadcast (-mu, rstd)(tb) -> [P,2,TBLK]: emitted one
                    # step AFTER b1(tb), so the contraction-1 matmuls' input
                    # rows are already resolved and the PE never stalls on
                    # the ACT/DVE row chain mid-FIFO
                    sr = statrow[tb]
                    bc = bcs.tile([P, 2, TBLK], BF16, tag="bc", name=f"bc{tb}")
                    if bc_dma:
                        nc.sync.dma_start(out=srow_d[tb], in_=sr)
                        nc.sync.dma_start(
                            out=bc,
                            in_=srow_d[tb].unsqueeze(0).to_broadcast([P, 2, TBLK]))
                    else:
                        bc_ps = pair_psum("bc", tb)
                        nc.tensor.matmul(bc_ps[:, 0, :], ones_row, sr[:, 0, :],
                                         start=True, stop=True)
                        nc.tensor.matmul(bc_ps[:, 1, :], ones_row, sr[:, 1, :],
                                         start=True, stop=True)
                        nc.vector.tensor_copy(out=bc, in_=bc_ps)
                    bc_t[tb] = bc

                # ---- BZ: LN apply -> fp8 lnp tiles ----
                lnp_t = [None] * NTB

                def z_block(tb):
                    xb = xb_t[tb]
                    bc = bc_t[tb]
                    # LN apply: za = x - mu (bf16 2x DVE); then one STT
                    # (za * gamma_col) * rstd_bc -> fp8 (beta==0 path);
                    # two of the four STTs go to GPSIMD.
                    lnp = []
                    for w in range(W2):
                        lnp.append(lns.tile([P, 2, TBLK], FP8, tag=f"lnp{w}",
                                            name=f"lnp{tb}_{w}"))
                    for cc in range(CB):
                        za = ztmp.tile([P, TBLK], BF16, tag="za", name=f"za{tb}_{cc}")
                        nc.vector.tensor_add(out=za, in0=xb[:, cc, :], in1=bc[:, 0, :])
                        dst = lnp[cc // 2][:, cc % 2, :]
                        if beta_zero and gamma_one:
                            # DVE is the loaded engine in phase B: give it
                            # only one of the four multiplies
                            eng = nc.gpsimd if (z_gpsimd and cc != 0) else nc.vector
                            eng.tensor_mul(out=dst, in0=za, in1=bc[:, 1, :])
                        elif beta_zero:
                            nc.vector.scalar_tensor_tensor(
                                out=dst, in0=za, scalar=gcolt[:, cc:cc + 1],
                                in1=bc[:, 1, :], op0=ALU.mult, op1=ALU.mult)
                        else:
                            zb = ztmp.tile([P, TBLK], BF16, tag="zb",
                                           name=f"zb{tb}_{cc}")
                            nc.vector.tensor_mul(out=zb, in0=za, in1=bc[:, 1, :])
                            nc.scalar.activation(out=dst, in_=zb, func=FP.Identity,
                                                 scale=gcolt[:, cc:cc + 1],
                                                 bias=bcolt[:, cc:cc + 1])
                    lnp_t[tb] = lnp

                # ---- B2: QKV (fp8 DoubleRow) ----
                def b2_block(tb):
                    ts = slice(tb * TBLK, (tb + 1) * TBLK)
                    lnp = lnp_t[tb]
                    # K^T: 2 psum pairs, each fed by 2 DR matmuls; evict ACT
                    for w in range(W2):
                        kp = pair_psum(f"kp{w}_", tb)
                        for j in range(2):
                            dd = 2 * w + j
                            for v in range(W2):
                                nc.tensor.matmul(
                                    kp[:, j, :],
                                    wq8[v][:, :, C + dd * P:C + (dd + 1) * P],
                                    lnp[v], perf_mode=DR,
                                    start=(v == 0), stop=(v == W2 - 1))
                        nc.scalar.activation(out=KT[w][:, :, ts], in_=kp, func=FP.Copy)
                    # V: 2 psum pairs (token-chunk pairs) -> resident V tiles;
                    # gpsimd can't read PSUM: split ACT/DVE (Q blocks put
                    # both on DVE since ACT carries QT there)
                    for m in range(2):
                        vp = pair_psum(f"vp{m}_", tb)
                        for j in range(2):
                            tt = 2 * m + j
                            for v in range(W2):
                                nc.tensor.matmul(
                                    vp[:, j, :],
                                    lnp[v][:, :, tt * P:(tt + 1) * P],
                                    wq8[v][:, :, 2 * C:3 * C], perf_mode=DR,
                                    start=(v == 0), stop=(v == W2 - 1))
                        if tb == NTB - 1 or (tb >= NQB and m == 0):
                            nc.scalar.activation(out=V[tb * 2 + m], in_=vp,
                                                 func=FP.Copy)
                        else:
                            nc.vector.tensor_copy(out=V[tb * 2 + m], in_=vp)
                    # Q^T (local queries only) -> resident QT tiles
                    if tb < NQB:
                        for w in range(W2):
                            qp = pair_psum(f"qp{w}_", tb)
                            for j in range(2):
                                dd = 2 * w + j
                                for v in range(W2):
                                    nc.tensor.matmul(
                                        qp[:, j, :],
                                        wq8[v][:, :, dd * P:(dd + 1) * P],
                                        lnp[v], perf_mode=DR,
                                        start=(v == 0), stop=(v == W2 - 1))
                            nc.scalar.activation(out=QT[tb][w], in_=qp,
                                                 func=FP.Copy)

                # 3-deep pipeline: stats+bc(s) | LN apply(s-1) | QKV(s-3)
                for step in range(NTB + 3):
                    if step < NTB:
                        b1_block(step)
                        bc_block(step)
                    if 1 <= step < NTB + 1:
                        z_block(step - 1)
                    if step >= 3:
                        b2_block(step - 3)

            # =========== Phase C: attention ===========
            with (
                tc.tile_pool(name="es", bufs=8) as es,
                tc.tile_pool(name="outts", bufs=2) as outts,
                tc.tile_pool(name="dens", bufs=2) as dens,
                tc.tile_pool(name="fins", bufs=2) as fins,
                tc.tile_pool(name="xrs", bufs=2) as xrs,
                tc.tile_pool(name="ps_s", bufs=3, space="PSUM") as ps_s,
                tc.tile_pool(name="ps_o", bufs=1, space="PSUM") as ps_o,
                tc.tile_pool(name="ps_d", bufs=1, space="PSUM") as ps_d,
            ):
                H2 = C // 2

                def make_tail(qb, outTp, den_ps, xr, last=False):
                    def tail():
                        den_row = dens.tile([1, TBLK], F32, tag="den_row",
                                            name=f"den_row{qb}")
                        nc.scalar.activation(out=den_row, in_=den_ps, func=FP.Copy)
                        # [1,512] -> [128,4] partition-major via 4 PE transposes
                        # (a DRAM roundtrip here costs ~5us of dead latency on
                        # the final tail)
                        den_pm = ps_d.tile([P, CB], F32, tag="pd",
                                           name=f"den_pm{qb}")
                        for qq in range(CB):
                            nc.tensor.transpose(den_pm[:, qq:qq + 1],
                                                den_row[:, qq * P:(qq + 1) * P],
                                                ones11)
                        recT = dens.tile([P, CB], F32, tag="recT", name=f"recT{qb}")
                        nc.vector.reciprocal(out=recT, in_=den_pm)
                        finb = fins.tile([P, CB, C], F32, tag="finb", name=f"finb{qb}")
                        # proj (fp8 DR) + normalize + residual per 128-query
                        # slice; STT + output DMA split into half-rows so the
                        # final store drains earlier
                        for qq in range(CB):
                            if last:
                                # qq 0-2 rotate the 3 pscr banks; qq3 takes
                                # the freed pop bank so it needn't wait for
                                # the qq0 fin STT to release a slot
                                if qq == CB - 1:
                                    pf = ps_o.tile([P, C], F32, tag="po0",
                                                   name=f"pf{qb}_{qq}")
                                else:
                                    pf = ps_s.tile([P, C], F32, tag="pscr",
                                                   name=f"pf{qb}_{qq}")
                            else:
                                pf = ps_d.tile([P, C], F32, tag="pd",
                                               name=f"pf{qb}_{qq}")
                            for w in range(W2):
                                nc.tensor.matmul(
                                    pf, outTp[w][:, :, qq * P:(qq + 1) * P],
                                    wp8[w], perf_mode=DR,
                                    start=(w == 0), stop=(w == W2 - 1))
                            rsl = slice(qb * TBLK + qq * P, qb * TBLK + (qq + 1) * P)
                            for h in range(2):
                                hsl = slice(h * H2, (h + 1) * H2)
                                nc.vector.scalar_tensor_tensor(
                                    out=finb[:, qq, hsl], in0=pf[:, hsl],
                                    scalar=recT[:, qq:qq + 1], in1=xr[:, qq, hsl],
                                    op0=ALU.mult, op1=ALU.add)
                                nc.sync.dma_start(out=out[rsl, hsl],
                                                  in_=finb[:, qq, hsl])
                    return tail

                pending_tail = None
                for qb in range(NQB):
                    qs = slice(qb * TBLK, (qb + 1) * TBLK)
                    # prefetch residual rows for this qb's tail
                    xr = xrs.tile([P, CB, C], F32, tag="xr", name=f"xr{qb}")
                    nc.sync.dma_start(
                        out=xr,
                        in_=xres[qs, :].rearrange("(qq p) c -> p qq c", qq=CB))
                    pop = [ps_o.tile([P, 2, TBLK], F32, tag=f"po{w}",
                                     name=f"po{qb}_{w}") for w in range(W2)]
                    dacc = dens.tile([P, 2, TBLK], F32, tag="dacc", name=f"dacc{qb}")
                    dacc2 = dens.tile([P, 2, TBLK], F32, tag="dacc2",
                                      name=f"dacc2_{qb}")
                    dacc_bf = dens.tile([P, 2, TBLK], BF16, tag="dacc_bf",
                                        name=f"dacc_bf{qb}")

                    pair_t = {}

                    def scores_exp(kt, qb=qb):
                        u = kt // 2
                        if kt % 2 == 0:
                            pair_t[u] = es.tile([P, 2, TBLK], FP8, tag="e",
                                                name=f"e{qb}_{u}")
                        ksl = slice(kt * P, (kt + 1) * P)
                        pscr = ps_s.tile([P, TBLK], F32, tag="pscr",
                                         name=f"pscr{qb}_{kt}")
                        for w in range(W2):
                            nc.tensor.matmul(pscr, KT[w][:, :, ksl], QT[qb][w],
                                             perf_mode=DR,
                                             start=(w == 0), stop=(w == W2 - 1))
                        # shifted exp (softmax-invariant) keeps E in fp8e4m3 range
                        nc.scalar.activation(out=pair_t[u][:, kt % 2, :], in_=pscr,
                                             func=FP.Exp, scale=SCALE / (WS * WS),
                                             bias=neg2)

                    scores_exp(0)
                    scores_exp(1)
                    for kt in range(NKT):
                        u = kt // 2
                        if kt + 2 < NKT:
                            scores_exp(kt + 2)
                        if kt % 2 == 1:
                            for cc in range(CB):
                                nc.tensor.matmul(
                                    pop[cc // 2][:, cc % 2, :],
                                    V[u][:, :, cc * P:(cc + 1) * P], pair_t[u],
                                    perf_mode=DR,
                                    start=(u == 0), stop=(u == NKT // 2 - 1))
                            # denominator partials for pairs u<=13 split over
                            # TWO accumulators - even pairs on DVE, odd pairs
                            # on the (C-phase-idle) GPSIMD - merged at u=13,
                            # so the serial 1.2us adds never back up the DVE
                            # FIFO at qb end; the last 2 pairs go via DR
                            # ones-matmuls below
                            if u == 0:
                                nc.vector.tensor_copy(out=dacc, in_=pair_t[u])
                            elif u == 1:
                                nc.gpsimd.tensor_copy(out=dacc2, in_=pair_t[u])
                            elif u < NKT // 2 - 2:
                                eng = nc.vector if u % 2 == 0 else nc.gpsimd
                                acc = dacc if u % 2 == 0 else dacc2
                                eng.tensor_add(out=acc, in0=acc, in1=pair_t[u])
                                if u == NKT // 2 - 3:
                                    nc.vector.tensor_add(out=dacc, in0=dacc,
                                                         in1=dacc2)
                                    nc.vector.tensor_copy(out=dacc_bf, in_=dacc)
                        if kt == 6 and pending_tail is not None:
                            pending_tail()
                            pending_tail = None
                    # partition-reduce: dacc_bf (u<=13, cast early) + the last
                    # two exp pairs directly via DR ones-matmuls
                    den_ps = ps_d.tile([1, TBLK], F32, tag="pd", name=f"den{qb}")
                    for j in range(2):
                        nc.tensor.matmul(den_ps, ones_col_bf, dacc_bf[:, j, :],
                                         start=(j == 0), stop=False)
                    nc.tensor.matmul(den_ps, ones_pair8, pair_t[NKT // 2 - 2],
                                     perf_mode=DR, start=False, stop=False)
                    nc.tensor.matmul(den_ps, ones_pair8, pair_t[NKT // 2 - 1],
                                     perf_mode=DR, start=False, stop=True)
                    # evict numerators to fp8 (scaled by 1/WS^2: pf comes out as
                    # num*wp, normalized by 1/den at the fin STT)
                    outTp = []
                    for w in range(W2):
                        t = outts.tile([P, 2, TBLK], FP8, tag=f"outT{w}",
                                       name=f"outT{qb}_{w}")
                        if w == 0:
                            nc.scalar.activation(out=t, in_=pop[w], func=FP.Copy,
                                                 scale=1.0 / (WS * WS))
                        else:
                            # ACT and DVE evict in parallel (matters on the
                            # last qb whose tail has nothing to hide behind)
                            nc.vector.tensor_scalar_mul(t, pop[w], 1.0 / (WS * WS))
                        outTp.append(t)
                    pending_tail = make_tail(qb, outTp, den_ps, xr,
                                             last=(qb == NQB - 1))
                if pending_tail is not None:
                    pending_tail()
    split_multiwaits(nc)
    return nc


_NC = {}


def kernel(x, ln_gamma, ln_beta, w_qkv, w_proj, **run_kwargs):
    import ml_dtypes
    x = np.ascontiguousarray(np.asarray(x, dtype=np.float32))
    ln_gamma = np.asarray(ln_gamma, dtype=np.float32)
    ln_beta = np.asarray(ln_beta, dtype=np.float32)
    fp8_np = mybir.dt.np(FP8)
    # channel-paired fp8 qkv weights, prescaled by WS:
    # wq8[w, p, j, d] = w_qkv[d, w*256 + j*128 + p] * WS
    wq8 = np.ascontiguousarray(
        (np.asarray(w_qkv, dtype=np.float32).T * WS)
        .reshape(W2, 2, P, 3 * C).transpose(0, 2, 1, 3)).astype(fp8_np)
    wp8 = np.ascontiguousarray(
        (np.asarray(w_proj, dtype=np.float32).T * WS)
        .reshape(W2, 2, P, C).transpose(0, 2, 1, 3)).astype(fp8_np)
    b, c, h, w = x.shape
    assert (b, c, h * w) == (4, C, T)

    beta_zero = not np.any(ln_beta)
    gamma_one = bool(np.all(ln_gamma == 1.0))

    in_maps = []
    for core in range(8):
        bi, half = core // 2, core % 2
        xt_b = x[bi].reshape(C, T)
        if half == 0:
            xt_i = xt_b
        else:
            xt_i = np.concatenate([xt_b[:, TQ:], xt_b[:, :TQ]], axis=1)
        xt_i = np.ascontiguousarray(xt_i)
        xres_i = np.ascontiguousarray(xt_i[:, :TQ].T)
        # fp8 pair tiles of x and x^2 for the DR stats matmuls:
        # x8q[p, w, j, s, t] = fp8(xt[w*256 + j*128 + p, t] ** (s+1))
        xr4 = xt_i.reshape(W2, 2, P, T).transpose(2, 0, 1, 3)  # [P, W2, 2, T]
        # tb-major: [P, NTB, W2, 2, 2, TBLK]
        x8q_i = np.ascontiguousarray(
            np.stack([xr4, xr4 * xr4], axis=3)
            .reshape(P, W2, 2, 2, NTB, TBLK).transpose(0, 4, 1, 2, 3, 5)
        ).astype(fp8_np)
        in_maps.append({
            "xbf": xt_i.astype(ml_dtypes.bfloat16),
            "x8q": x8q_i,
            "xres": xres_i, "wq8d": wq8, "wp8d": wp8,
            "gamma": ln_gamma, "beta": ln_beta,
        })

    key = (beta_zero, gamma_one)
    if key not in _NC:
        _NC[key] = build_nc(beta_zero=beta_zero, gamma_one=gamma_one)
    res = run_bass_kernel_spmd(_NC[key], in_maps, core_ids=list(range(8)),
                               **run_kwargs)

    y = np.empty((b, T, C), dtype=np.float32)
    for core in range(8):
        bi, half = core // 2, core % 2
        y[bi, half * TQ:(half + 1) * TQ, :] = res.results[core]["out"]
    y = np.ascontiguousarray(y.transpose(0, 2, 1).reshape(b, C, h, w))
    if run_kwargs:
        return y, res
    return y


# revision 33
# speedup vs baseline: 1.0470x; 1.0470x over previous
"""AttnBlock (LayerNorm + single-head self-attention + proj + residual) on 8
Trainium2 NeuronCores.

Problem: x [4, 512, 64, 64] f32; per batch image: t = LN(x) over channels;
qkv = t @ w_qkv.T; attn = softmax(q k^T / sqrt(c)); out = attn v @ w_proj.T;
y = x + out.

Sharding: 8 cores = 4 batches x 2 query-halves. Each core gets its batch's
full image (token order rolled so its 2048 queries are local tokens 0..2047),
computes LN + K/V over all 4096 tokens and Q over its half, then
scores/softmax/attn-V/proj for its 2048 queries. No collectives.

v4 design notes (vs v3):
  - Phase B pipeline deepened to 3 steps (stats -> LN apply -> QKV) so the
    serial stats->broadcast->apply chain (~7us with sem hops) never stalls
    the PE's QKV matmuls.
  - The -mu/rstd broadcast is two chained DMAs (row to DRAM, then a
    partition-broadcast read back): no PE matmuls, no DVE psum-copy.
  - LN stats: fp8 DR ones-matmuls over host-marshaled fp8 pair tiles of x
    and x^2; block 0's tile is split into 4 chunk DMAs so the first stats
    matmul starts after 128KB, not 512KB.
  - LN apply: za = x - mu (DVE 2x), then one scalar_tensor_tensor
    (za * gamma_col) * rstd_bc -> fp8; two of the four casts per block run
    on GPSIMD. beta==0 fast path (general beta build falls back).
  - Evictions: K^T/Q^T on ACT, V split ACT/DVE (gpsimd can't read PSUM).
  - Softmax denominator: DVE accumulates exp pairs u<=13 (cast early);
    pairs 14/15 are reduced by two DR ones-matmuls straight into the den
    PSUM, so the last tail never waits on the serial DVE add chain.
    es pool holds 4 exp pairs so score prefetch doesn't block on DVE lag.
  - fin STT + output DMA split into half-rows; out/xr/xbf DMAs issue on
    the SP hardware DGE (gpsimd keeps the 5D fp8 stat loads).
"""
import numpy as np

import concourse.bass as bass
import concourse.tile as tile
from concourse import mybir
from concourse.bass_utils import run_bass_kernel_spmd

P = 128
C = 512          # channels
T = 4096         # tokens per image
TQ = 2048        # queries per core
CB = C // P      # 4 channel chunks
W2 = CB // 2     # 2 channel chunk-pairs
TBLK = 512       # token block for LN/QKV phase
NTB = T // TBLK  # 8
NQB = TQ // TBLK  # 4 query blocks
NKT = T // P     # 32 key chunks
F32 = mybir.dt.float32
BF16 = mybir.dt.bfloat16
FP8 = mybir.dt.float8e4
FP = mybir.ActivationFunctionType
ALU = mybir.AluOpType
DR = mybir.MatmulPerfMode.DoubleRow
SCALE = float(C) ** -0.5
WS = 8.0          # fp8 weight prescale


def split_multiwaits(nc, max_waits=1):
    """walrus codegen allows one sync-wait slot on most TPB instruction
    structs; Tile's sem assignment emits several. Split extras into
    wait-only EventSemaphore instructions on the same engine stream."""
    n = 0
    for fn in nc.m.functions:
        for blk in fn.blocks:
            out = []
            for inst in blk.instructions:
                si = inst.sync_info
                if si is not None and si.on_wait is not None and len(si.on_wait) > max_waits:
                    extra = list(si.on_wait[:-max_waits])
                    keep = list(si.on_wait[-max_waits:])
                    for w in extra:
                        ev = mybir.InstEventSemaphore(
                            name=nc.get_next_instruction_name(),
                            engine=inst.engine,
                            sync_info=mybir.SyncInfo(on_wait=[w], on_update=[]),
                        )
                        out.append(ev)
                        n += 1
                    si.on_wait = keep
                out.append(inst)
            blk.instructions[:] = out
    return n


def build_nc(beta_zero=True, gamma_one=True, z_gpsimd=True, bc_dma=False):
    """beta_zero: ln_beta==0 (host-verified) - LN apply fuses gamma*rstd
    into one STT writing fp8. gamma_one: ln_gamma==1 as well - the STT
    becomes a plain tensor_mul, which Pool supports (TensorScalarPtr is
    rejected on Pool), so two of four casts can run on GPSIMD (z_gpsimd).
    bc_dma: broadcast -mu/rstd via DRAM bounce instead of PE matmuls."""
    nc = bass.Bass()
    xbf = nc.declare_dram_parameter("xbf", [C, T], BF16, isOutput=False)
    # tb-major fp8 stat tiles: each [:, tb] block is per-partition contiguous
    # (4KB), so the loads are 2D/3D patterns the SP hardware DGE can issue -
    # gpsimd's queue stays empty for the LN-apply multiplies
    x8q = nc.declare_dram_parameter("x8q", [P, NTB, W2, 2, 2, TBLK], FP8,
                                    isOutput=False)
    xres = nc.declare_dram_parameter("xres", [TQ, C], F32, isOutput=False)
    wq8d = nc.declare_dram_parameter("wq8d", [W2, P, 2, 3 * C], FP8, isOutput=False)
    wp8d = nc.declare_dram_parameter("wp8d", [W2, P, 2, C], FP8, isOutput=False)
    gamma = nc.declare_dram_parameter("gamma", [C], F32, isOutput=False)
    beta = nc.declare_dram_parameter("beta", [C], F32, isOutput=False)
    out = nc.declare_dram_parameter("out", [TQ, C], F32, isOutput=True)
    srow_d = nc.dram_tensor("srow_d", [NTB, 2, TBLK], BF16)

    with tile.TileContext(nc) as tc:
        with (
            tc.tile_pool(name="xs", bufs=3) as xs,
            tc.tile_pool(name="x8s", bufs=3) as x8s,
            tc.tile_pool(name="consts", bufs=1) as consts,
            tc.tile_pool(name="resid", bufs=1) as resid,
        ):
            # first gpsimd issues: block 0's fp8 stat tile, split by
            # (w, x|sq) chunk so the first stats matmul waits on 128KB
            x8_0 = consts.tile([P, W2, 2, 2, TBLK], FP8, tag="x8_0", name="x8_0")
            for w in range(W2):
                for s in range(2):
                    nc.sync.dma_start(out=x8_0[:, w, :, s, :],
                                      in_=x8q[:, 0, w, :, s, :])
            xb0 = consts.tile([P, CB, TBLK], BF16, tag="xb0", name="xb0")
            nc.sync.dma_start(
                out=xb0, in_=xbf[:, 0:TBLK].rearrange("(cb p) t -> p cb t", cb=CB))
            x8_1 = x8s.tile([P, W2, 2, 2, TBLK], FP8, tag="x8", name="x8_1pre")
            nc.sync.dma_start(out=x8_1, in_=x8q[:, 1])
            xb1_pre = xs.tile([P, CB, TBLK], BF16, tag="xb", name="xb1_pre")
            nc.sync.dma_start(
                out=xb1_pre,
                in_=xbf[:, TBLK:2 * TBLK].rearrange("(cb p) t -> p cb t", cb=CB))
            wq8 = []
            for w in range(W2):
                t = consts.tile([P, 2, 3 * C], FP8, tag=f"wq8_{w}", name=f"wq8_{w}")
                nc.sync.dma_start(out=t, in_=wq8d[w])
                wq8.append(t)
            # ---- constants ----
            gcolt = consts.tile([P, CB], F32, tag="gcolt")
            nc.sync.dma_start(out=gcolt, in_=gamma.rearrange("(cb p) -> p cb", cb=CB))
            bcolt = consts.tile([P, CB], F32, tag="bcolt")
            nc.sync.dma_start(out=bcolt, in_=beta.rearrange("(cb p) -> p cb", cb=CB))
            wp8 = []
            for w in range(W2):
                t = consts.tile([P, 2, C], FP8, tag=f"wp8_{w}", name=f"wp8_{w}")
                nc.sync.dma_start(out=t, in_=wp8d[w])
                wp8.append(t)
            ones_col_bf = consts.tile([P, 1], BF16, tag="ones_col_bf")
            nc.vector.memset(ones_col_bf, 1.0)
            ones_row = consts.tile([1, P], BF16, tag="ones_row")
            nc.vector.memset(ones_row, 1.0)
            # stats/den stationary: [128,2,1] slice of a [128,2,16] tile
            # (dual-fp8 LDWEIGHTS rejects a free-standing [128,2,1] with
            # tiny row stride; 16-elem pair stride satisfies step%16==0)
            ones_pair_w = consts.tile([P, 2, 16], FP8, tag="ones_pair_w")
            nc.vector.memset(ones_pair_w, 1.0)
            ones_pair8 = ones_pair_w[:, :, 0:1]
            eps_t = consts.tile([1, 1], F32, tag="eps_t")
            nc.vector.memset(eps_t, 1e-5)
            ones11 = consts.tile([1, 1], F32, tag="ones11")
            nc.vector.memset(ones11, 1.0)
            neg2 = consts.tile([P, 1], F32, tag="neg2")
            nc.vector.memset(neg2, -2.0)

            # ---- resident tensors ----
            KT = []   # K^T pairs: 2 x [128, 2, 4096] fp8 (DoubleRow layout)
            for w in range(W2):
                KT.append(resid.tile([P, 2, T], FP8, tag=f"KT{w}", name=f"KT{w}"))
            V = []    # V [tokenpair, d]: 16 x [128, 2, 512] fp8 (DoubleRow layout)
            for u in range(NKT // 2):
                V.append(resid.tile([P, 2, C], FP8, tag=f"V{u}", name=f"V{u}"))
            QT = []   # resident Q: per qb, per w: [128, 2, 512] fp8
            for qb in range(NQB):
                QT.append([resid.tile([P, 2, TBLK], FP8, tag=f"QT{qb}_{w}",
                                      name=f"QT{qb}_{w}") for w in range(W2)])

            # =========== Phase B: LN + QKV ===========
            with (
                tc.tile_pool(name="stat", bufs=1) as stat,
                tc.tile_pool(name="rows", bufs=2) as rows,
                tc.tile_pool(name="bcs", bufs=3) as bcs,
                tc.tile_pool(name="lns", bufs=4) as lns,
                tc.tile_pool(name="ztmp", bufs=3) as ztmp,
                tc.tile_pool(name="ps_row", bufs=1, space="PSUM") as ps_row,
                tc.tile_pool(name="ps_q", bufs=1, space="PSUM") as ps_q,
            ):
                statrow = [None] * NTB   # [1,2,TBLK] bf16: (-mu, rstd) rows
                bc_t = [None] * NTB
                xb_t = [None] * NTB
                x8_t = [None] * NTB
                q_slot = [0]

                def pair_psum(prefix, tb):
                    tag = f"pq{q_slot[0] % 3}"
                    q_slot[0] += 1
                    return ps_q.tile([P, 2, TBLK], F32, tag=tag,
                                     name=f"{prefix}{tb}")

                def load_xb(tb):
                    if tb >= NTB or xb_t[tb] is not None:
                        return
                    if tb == 0:
                        xb_t[tb] = xb0
                        x8_t[tb] = x8_0
                        return
                    if tb == 1:
                        xb_t[tb] = xb1_pre
                        x8_t[tb] = x8_1
                        return
                    ts = slice(tb * TBLK, (tb + 1) * TBLK)
                    x8 = x8s.tile([P, W2, 2, 2, TBLK], FP8, tag="x8",
                                  name=f"x8_{tb}")
                    nc.sync.dma_start(out=x8, in_=x8q[:, tb])
                    x8_t[tb] = x8
                    xb = xs.tile([P, CB, TBLK], BF16, tag="xb", name=f"xb{tb}")
                    nc.sync.dma_start(
                        out=xb, in_=xbf[:, ts].rearrange("(cb p) t -> p cb t", cb=CB))
                    xb_t[tb] = xb

                def b1_block(tb):
                    load_xb(tb)
                    load_xb(tb + 1)  # prefetch next block's x a step early
                    x8 = x8_t[tb]
                    # stats: fp8 DR ones-matmuls (contraction 256/pass)
                    s1 = ps_row.tile([1, TBLK], F32, tag="s1", name=f"s1_{tb}")
                    for w in range(W2):
                        nc.tensor.matmul(s1, ones_pair8, x8[:, w, :, 0, :],
                                         perf_mode=DR,
                                         start=(w == 0), stop=(w == W2 - 1))
                    s2 = ps_row.tile([1, TBLK], F32, tag="s2", name=f"s2_{tb}")
                    for w in range(W2):
                        nc.tensor.matmul(s2, ones_pair8, x8[:, w, :, 1, :],
                                         perf_mode=DR,
                                         start=(w == 0), stop=(w == W2 - 1))
                    # -mu row (ACT), then mu^2 and var on GPSIMD (immediate
                    # scalars only - Pool rejects scalar pointers), keeping
                    # DVE free for the LN apply and evictions
                    sr = stat.tile([1, 2, TBLK], BF16, tag=f"statrow{tb}",
                                   name=f"statrow{tb}")
                    nc.scalar.activation(out=sr[:, 0, :], in_=s1, func=FP.Copy,
                                         scale=-1.0 / C)
                    musq = rows.tile([1, TBLK], F32, tag="musq", name=f"musq{tb}")
                    nc.vector.tensor_mul(out=musq, in0=sr[:, 0, :], in1=sr[:, 0, :])
                    var = rows.tile([1, TBLK], F32, tag="var", name=f"var{tb}")
                    nc.vector.scalar_tensor_tensor(
                        out=var, in0=s2, scalar=1.0 / C, in1=musq,
                        op0=ALU.mult, op1=ALU.subtract)
                    lnv = rows.tile([1, TBLK], F32, tag="lnv", name=f"lnv{tb}")
                    nc.scalar.activation(out=lnv, in_=var, func=FP.Ln, bias=eps_t)
                    nc.scalar.activation(out=sr[:, 1, :], in_=lnv, func=FP.Exp,
                                         scale=-0.5)
                    statrow[tb] = sr

                def bc_block(tb):
                    # broadcast (-mu, rstd)(tb) -> [P,2,TBLK]: emitted one
                    # step AFTER b1(tb), so the contraction-1 matmuls' input
                    # rows are already resolved and the PE never stalls on
                    # the ACT/DVE row chain mid-FIFO
                    sr = statrow[tb]
                    bc = bcs.tile([P, 2, TBLK], BF16, tag="bc", name=f"bc{tb}")
                    if bc_dma:
                        nc.sync.dma_start(out=srow_d[tb], in_=sr)
                        nc.sync.dma_start(
                            out=bc,
                            in_=srow_d[tb].unsqueeze(0).to_broadcast([P, 2, TBLK]))
                    else:
                        bc_ps = pair_psum("bc", tb)
                        nc.tensor.matmul(bc_ps[:, 0, :], ones_row, sr[:, 0, :],
                                         start=True, stop=True)
                        nc.tensor.matmul(bc_ps[:, 1, :], ones_row, sr[:, 1, :],
                                         start=True, stop=True)
                        nc.vector.tensor_copy(out=bc, in_=bc_ps)
                    bc_t[tb] = bc

                # ---- BZ: LN apply -> fp8 lnp tiles ----
                lnp_t = [None] * NTB

                def z_block(tb):
                    xb = xb_t[tb]
                    bc = bc_t[tb]
                    # LN apply: za = x - mu (bf16 2x DVE); then one STT
                    # (za * gamma_col) * rstd_bc -> fp8 (beta==0 path);
                    # two of the four STTs go to GPSIMD.
                    lnp = []
                    for w in range(W2):
                        lnp.append(lns.tile([P, 2, TBLK], FP8, tag=f"lnp{w}",
                                            name=f"lnp{tb}_{w}"))
                    for cc in range(CB):
                        za = ztmp.tile([P, TBLK], BF16, tag="za", name=f"za{tb}_{cc}")
                        nc.vector.tensor_add(out=za, in0=xb[:, cc, :], in1=bc[:, 0, :])
                        dst = lnp[cc // 2][:, cc % 2, :]
                        if beta_zero and gamma_one:
                            # DVE is the loaded engine in phase B: give it
                            # only one of the four multiplies
                            eng = nc.gpsimd if (z_gpsimd and cc != 0) else nc.vector
                            eng.tensor_mul(out=dst, in0=za, in1=bc[:, 1, :])
                        elif beta_zero:
                            nc.vector.scalar_tensor_tensor(
                                out=dst, in0=za, scalar=gcolt[:, cc:cc + 1],
                                in1=bc[:, 1, :], op0=ALU.mult, op1=ALU.mult)
                        else:
                            zb = ztmp.tile([P, TBLK], BF16, tag="zb",
                                           name=f"zb{tb}_{cc}")
                            nc.vector.tensor_mul(out=zb, in0=za, in1=bc[:, 1, :])
                            nc.scalar.activation(out=dst, in_=zb, func=FP.Identity,
                                                 scale=gcolt[:, cc:cc + 1],
                                                 bias=bcolt[:, cc:cc + 1])
                    lnp_t[tb] = lnp

                # ---- B2: QKV (fp8 DoubleRow) ----
                def b2_block(tb):
                    ts = slice(tb * TBLK, (tb + 1) * TBLK)
                    lnp = lnp_t[tb]
                    # K^T: 2 psum pairs, each fed by 2 DR matmuls; evict ACT
                    for w in range(W2):
                        kp = pair_psum(f"kp{w}_", tb)
                        for j in range(2):
                            dd = 2 * w + j
                            for v in range(W2):
                                nc.tensor.matmul(
                                    kp[:, j, :],
                                    wq8[v][:, :, C + dd * P:C + (dd + 1) * P],
                                    lnp[v], perf_mode=DR,
                                    start=(v == 0), stop=(v == W2 - 1))
                        nc.scalar.activation(out=KT[w][:, :, ts], in_=kp, func=FP.Copy)
                    # V: 2 psum pairs (token-chunk pairs) -> resident V tiles;
                    # gpsimd can't read PSUM: split ACT/DVE (Q blocks put
                    # both on DVE since ACT carries QT there)
                    for m in range(2):
                        vp = pair_psum(f"vp{m}_", tb)
                        for j in range(2):
                            tt = 2 * m + j
                            for v in range(W2):
                                nc.tensor.matmul(
                                    vp[:, j, :],
                                    lnp[v][:, :, tt * P:(tt + 1) * P],
                                    wq8[v][:, :, 2 * C:3 * C], perf_mode=DR,
                                    start=(v == 0), stop=(v == W2 - 1))
                        if tb == NTB - 1 or (tb >= NQB and m == 0):
                            nc.scalar.activation(out=V[tb * 2 + m], in_=vp,
                                                 func=FP.Copy)
                        else:
                            nc.vector.tensor_copy(out=V[tb * 2 + m], in_=vp)
                    # Q^T (local queries only) -> resident QT tiles
                    if tb < NQB:
                        for w in range(W2):
                            qp = pair_psum(f"qp{w}_", tb)
                            for j in range(2):
                                dd = 2 * w + j
                                for v in range(W2):
                                    nc.tensor.matmul(
                                        qp[:, j, :],
                                        wq8[v][:, :, dd * P:(dd + 1) * P],
                                        lnp[v], perf_mode=DR,
                                        start=(v == 0), stop=(v == W2 - 1))
                            nc.scalar.activation(out=QT[tb][w], in_=qp,
                                                 func=FP.Copy)

                # 3-deep pipeline: stats+bc(s) | LN apply(s-1) | QKV(s-3)
                for step in range(NTB + 3):
                    if step < NTB:
                        b1_block(step)
                        bc_block(step)
                    if 1 <= step < NTB + 1:
                        z_block(step - 1)
                    if step >= 3:
                        b2_block(step - 3)

            # =========== Phase C: attention ===========
            with (
                tc.tile_pool(name="es", bufs=8) as es,
                tc.tile_pool(name="outts", bufs=2) as outts,
                tc.tile_pool(name="dens", bufs=2) as dens,
                tc.tile_pool(name="fins", bufs=2) as fins,
                tc.tile_pool(name="xrs", bufs=2) as xrs,
                tc.tile_pool(name="ps_s", bufs=3, space="PSUM") as ps_s,
                tc.tile_pool(name="ps_o", bufs=1, space="PSUM") as ps_o,
                tc.tile_pool(name="ps_d", bufs=1, space="PSUM") as ps_d,
            ):
                H2 = C // 2

                def make_tail(qb, outTp, den_ps, xr, last=False):
                    def tail():
                        den_row = dens.tile([1, TBLK], F32, tag="den_row",
                                            name=f"den_row{qb}")
                        nc.scalar.activation(out=den_row, in_=den_ps, func=FP.Copy)
                        # [1,512] -> [128,4] partition-major via 4 PE transposes
                        # (a DRAM roundtrip here costs ~5us of dead latency on
                        # the final tail)
                        den_pm = ps_d.tile([P, CB], F32, tag="pd",
                                           name=f"den_pm{qb}")
                        for qq in range(CB):
                            nc.tensor.transpose(den_pm[:, qq:qq + 1],
                                                den_row[:, qq * P:(qq + 1) * P],
                                                ones11)
                        recT = dens.tile([P, CB], F32, tag="recT", name=f"recT{qb}")
                        nc.vector.reciprocal(out=recT, in_=den_pm)
                        finb = fins.tile([P, CB, C], F32, tag="finb", name=f"finb{qb}")
                        # proj (fp8 DR) + normalize + residual per 128-query
                        # slice; STT + output DMA split into half-rows so the
                        # final store drains earlier
                        for qq in range(CB):
                            if last:
                                # qq 0-2 rotate the 3 pscr banks; qq3 takes
                                # the freed pop bank so it needn't wait for
                                # the qq0 fin STT to release a slot
                                if qq == CB - 1:
                                    pf = ps_o.tile([P, C], F32, tag="po0",
                                                   name=f"pf{qb}_{qq}")
                                else:
                                    pf = ps_s.tile([P, C], F32, tag="pscr",
                                                   name=f"pf{qb}_{qq}")
                            else:
                                pf = ps_d.tile([P, C], F32, tag="pd",
                                               name=f"pf{qb}_{qq}")
                            for w in range(W2):
                                nc.tensor.matmul(
                                    pf, outTp[w][:, :, qq * P:(qq + 1) * P],
                                    wp8[w], perf_mode=DR,
                                    start=(w == 0), stop=(w == W2 - 1))
                            rsl = slice(qb * TBLK + qq * P, qb * TBLK + (qq + 1) * P)
                            for h in range(2):
                                hsl = slice(h * H2, (h + 1) * H2)
                                nc.vector.scalar_tensor_tensor(
                                    out=finb[:, qq, hsl], in0=pf[:, hsl],
                                    scalar=recT[:, qq:qq + 1], in1=xr[:, qq, hsl],
                                    op0=ALU.mult, op1=ALU.add)
                                nc.sync.dma_start(out=out[rsl, hsl],
                                                  in_=finb[:, qq, hsl])
                    return tail

                pending_tail = None
                for qb in range(NQB):
                    qs = slice(qb * TBLK, (qb + 1) * TBLK)
                    # prefetch residual rows for this qb's tail
                    xr = xrs.tile([P, CB, C], F32, tag="xr", name=f"xr{qb}")
                    nc.sync.dma_start(
                        out=xr,
                        in_=xres[qs, :].rearrange("(qq p) c -> p qq c", qq=CB))
                    pop = [ps_o.tile([P, 2, TBLK], F32, tag=f"po{w}",
                                     name=f"po{qb}_{w}") for w in range(W2)]
                    dacc = dens.tile([P, 2, TBLK], F32, tag="dacc", name=f"dacc{qb}")
                    dacc2 = dens.tile([P, 2, TBLK], F32, tag="dacc2",
                                      name=f"dacc2_{qb}")
                    dacc_bf = dens.tile([P, 2, TBLK], BF16, tag="dacc_bf",
                                        name=f"dacc_bf{qb}")

                    pair_t = {}

                    def scores_exp(kt, qb=qb):
                        u = kt // 2
                        if kt % 2 == 0:
                            pair_t[u] = es.tile([P, 2, TBLK], FP8, tag="e",
                                                name=f"e{qb}_{u}")
                        ksl = slice(kt * P, (kt + 1) * P)
                        pscr = ps_s.tile([P, TBLK], F32, tag="pscr",
                                         name=f"pscr{qb}_{kt}")
                        for w in range(W2):
                            nc.tensor.matmul(pscr, KT[w][:, :, ksl], QT[qb][w],
                                             perf_mode=DR,
                                             start=(w == 0), stop=(w == W2 - 1))
                        # shifted exp (softmax-invariant) keeps E in fp8e4m3 range
                        nc.scalar.activation(out=pair_t[u][:, kt % 2, :], in_=pscr,
                                             func=FP.Exp, scale=SCALE / (WS * WS),
                                             bias=neg2)

                    scores_exp(0)
                    scores_exp(1)
                    for kt in range(NKT):
                        u = kt // 2
                        if kt + 2 < NKT:
                            scores_exp(kt + 2)
                        if kt % 2 == 1:
                            for cc in range(CB):
                                nc.tensor.matmul(
                                    pop[cc // 2][:, cc % 2, :],
                                    V[u][:, :, cc * P:(cc + 1) * P], pair_t[u],
                                    perf_mode=DR,
                                    start=(u == 0), stop=(u == NKT // 2 - 1))
                            # denominator partials for pairs u<=13 split over
                            # TWO accumulators - even pairs on DVE, odd pairs
                            # on the (C-phase-idle) GPSIMD - merged at u=13,
                            # so the serial 1.2us adds never back up the DVE
                            # FIFO at qb end; the last 2 pairs go via DR
                            # ones-matmuls below
                            if u == 0:
                                nc.vector.tensor_copy(out=dacc, in_=pair_t[u])
                            elif u == 1:
                                nc.gpsimd.tensor_copy(out=dacc2, in_=pair_t[u])
                            elif u < NKT // 2 - 2:
                                eng = nc.vector if u % 2 == 0 else nc.gpsimd
                                acc = dacc if u % 2 == 0 else dacc2
                                eng.tensor_add(out=acc, in0=acc, in1=pair_t[u])
                                if u == NKT // 2 - 3:
                                    nc.vector.tensor_add(out=dacc, in0=dacc,
                                                         in1=dacc2)
                                    nc.vector.tensor_copy(out=dacc_bf, in_=dacc)
                        if kt == 6 and pending_tail is not None:
                            pending_tail()
                            pending_tail = None
                    # partition-reduce: dacc_bf (u<=13, cast early) + the last
                    # two exp pairs directly via DR ones-matmuls
                    den_ps = ps_d.tile([1, TBLK], F32, tag="pd", name=f"den{qb}")
                    for j in range(2):
                        nc.tensor.matmul(den_ps, ones_col_bf, dacc_bf[:, j, :],
                                         start=(j == 0), stop=False)
                    nc.tensor.matmul(den_ps, ones_pair8, pair_t[NKT // 2 - 2],
                                     perf_mode=DR, start=False, stop=False)
                    nc.tensor.matmul(den_ps, ones_pair8, pair_t[NKT // 2 - 1],
                                     perf_mode=DR, start=False, stop=True)
                    # evict numerators to fp8 (scaled by 1/WS^2: pf comes out as
                    # num*wp, normalized by 1/den at the fin STT)
                    outTp = []
                    for w in range(W2):
                        t = outts.tile([P, 2, TBLK], FP8, tag=f"outT{w}",
                                       name=f"outT{qb}_{w}")
                        if w == 0:
                            nc.scalar.activation(out=t, in_=pop[w], func=FP.Copy,
                                                 scale=1.0 / (WS * WS))
                        else:
                            # ACT and DVE evict in parallel (matters on the
                            # last qb whose tail has nothing to hide behind)
                            nc.vector.tensor_scalar_mul(t, pop[w], 1.0 / (WS * WS))
                        outTp.append(t)
                    pending_tail = make_tail(qb, outTp, den_ps, xr,
                                             last=(qb == NQB - 1))
                if pending_tail is not None:
                    pending_tail()
    split_multiwaits(nc)
    return nc


_NC = {}


def kernel(x, ln_gamma, ln_beta, w_qkv, w_proj, **run_kwargs):
    import ml_dtypes
    x = np.ascontiguousarray(np.asarray(x, dtype=np.float32))
    ln_gamma = np.asarray(ln_gamma, dtype=np.float32)
    ln_beta = np.asarray(ln_beta, dtype=np.float32)
    fp8_np = mybir.dt.np(FP8)
    # channel-paired fp8 qkv weights, prescaled by WS:
    # wq8[w, p, j, d] = w_qkv[d, w*256 + j*128 + p] * WS
    wq8 = np.ascontiguousarray(
        (np.asarray(w_qkv, dtype=np.float32).T * WS)
        .reshape(W2, 2, P, 3 * C).transpose(0, 2, 1, 3)).astype(fp8_np)
    wp8 = np.ascontiguousarray(
        (np.asarray(w_proj, dtype=np.float32).T * WS)
        .reshape(W2, 2, P, C).transpose(0, 2, 1, 3)).astype(fp8_np)
    b, c, h, w = x.shape
    assert (b, c, h * w) == (4, C, T)

    beta_zero = not np.any(ln_beta)
    gamma_one = bool(np.all(ln_gamma == 1.0))

    in_maps = []
    for core in range(8):
        bi, half = core // 2, core % 2
        xt_b = x[bi].reshape(C, T)
        if half == 0:
            xt_i = xt_b
        else:
            xt_i = np.concatenate([xt_b[:, TQ:], xt_b[:, :TQ]], axis=1)
        xt_i = np.ascontiguousarray(xt_i)
        xres_i = np.ascontiguousarray(xt_i[:, :TQ].T)
        # fp8 pair tiles of x and x^2 for the DR stats matmuls:
        # x8q[p, w, j, s, t] = fp8(xt[w*256 + j*128 + p, t] ** (s+1))
        xr4 = xt_i.reshape(W2, 2, P, T).transpose(2, 0, 1, 3)  # [P, W2, 2, T]
        # tb-major: [P, NTB, W2, 2, 2, TBLK]
        x8q_i = np.ascontiguousarray(
            np.stack([xr4, xr4 * xr4], axis=3)
            .reshape(P, W2, 2, 2, NTB, TBLK).transpose(0, 4, 1, 2, 3, 5)
        ).astype(fp8_np)
        in_maps.append({
            "xbf": xt_i.astype(ml_dtypes.bfloat16),
            "x8q": x8q_i,
            "xres": xres_i, "wq8d": wq8, "wp8d": wp8,
            "gamma": ln_gamma, "beta": ln_beta,
        })

    key = (beta_zero, gamma_one)
    if key not in _NC:
        _NC[key] = build_nc(beta_zero=beta_zero, gamma_one=gamma_one)
    res = run_bass_kernel_spmd(_NC[key], in_maps, core_ids=list(range(8)),
                               **run_kwargs)

    y = np.empty((b, T, C), dtype=np.float32)
    for core in range(8):
        bi, half = core // 2, core % 2
        y[bi, half * TQ:(half + 1) * TQ, :] = res.results[core]["out"]
    y = np.ascontiguousarray(y.transpose(0, 2, 1).reshape(b, C, h, w))
    if run_kwargs:
        return y, res
    return y
